# revision 6
# baseline (speedup 1.0000x reference)
"""FocalLoss + MDCA loss kernel for TRN2, 8-core data-parallel. v2.

reference:
    loss_cls = mean_i[-(1-pt_i) * log(pt_i)],  pt_i = probs[i, targets[i]]
    loss_cal = mean_c |mean_i probs[i,c] - count_c/B|
    out = loss_cls + loss_cal        (GAMMA=1, BETA=1)

Strategy: shard batch (16384) across 8 cores (2048 rows each). Each core:
  - streams its probs shard HBM->SBUF as plain fp32 via HWDGE on the SP
    ring (no Q7 descriptor-gen in the stream path, no SWDGE engine-7/15
    descriptor-ring penalty); targets ride the other HWDGE ring (ACT)
  - per-tile fp32->fp16 casts run on ACT/GpSimd (DVE takes the last two
    so the final tile clears the cast in ~0.5us), DMA-paced
  - PE matmul ones[128,1]^T @ cast_fp16 accumulates column sums in PSUM
  - DVE builds one-hot rows eq[p,c] = (c == target_p) from an iota
    constant; PE matmul ones^T @ eq accumulates the target histogram
  - pt[p, j] = probs[128j+p, t] via ONE indirect SWDGE gather (exact
    fp32); the whole focal chain (ACT stages [pt|ln pt], DVE fused
    (pt-1)*ln(pt) row-sum, PE transpose, ACT accumulate) completes
    mid-stream, off the critical tail
  - tail after the last probs byte: cast -> 2 matmuls -> parallel PSUM
    drains (DVE half / ACT half) -> one [1,2001] output DMA
Host combines the 8 cores' colsum/hist/focal partials into the scalar
loss (the gather/unshard step).

The walrus build in this env encodes at most ONE sync wait per instruction;
_split_multi_waits post-processes the scheduled program to hoist extra waits
onto same-engine EventSemaphore carriers.

The walrus codegen epilogue clears every semaphore id up to its allocation
cap (default 256) with one EVENT_SEMAPHORE each, split across engines —
~6us of cleanup inside the profiled window. Nothing in this program touches
a sem above ~175, so we cap the allocator (--max-sem-num) and the clear
loop shrinks with it.
"""

import numpy as np

import concourse.bass as bass
import concourse.bass_utils as _bu
import concourse.mybir as mybir
import concourse.tile as tile
from concourse.bass_utils import run_bass_kernel_spmd

if not getattr(_bu.bir_verify_and_optimise, "_sem_capped", False):
    _orig_bvo = _bu.bir_verify_and_optimise

    def _bvo_capped(*args, **kwargs):
        import concourse.bass_utils as bu

        orig_run = bu.run_command

        def run_with_cap(cmd, **kw):
            if any("codegen" in str(c) for c in cmd):
                cmd = list(cmd) + ["--max-sem-num=32"]
            return orig_run(cmd, **kw)

        bu.run_command = run_with_cap
        try:
            return _orig_bvo(*args, **kwargs)
        finally:
            bu.run_command = orig_run

    _bvo_capped._sem_capped = True
    _bu.bir_verify_and_optimise = _bvo_capped

B, C = 16384, 1000
NCORES = 8
BC = B // NCORES  # 2048 rows per core
P = 128
NT = BC // P      # 16 batch tiles per core
CH = 500          # matmul half free-dim (PSUM bank = 512 fp32)
OUT_W = 2001      # [colsum 0:1000 | hist 1000:2000 | focal_sum 2000]

F32 = mybir.dt.float32
F16 = mybir.dt.float16
I16 = mybir.dt.int16
I32 = mybir.dt.int32

# which engine casts tile i to fp16: ACT and GpSimd alternate through the
# DMA-paced middle, DVE (fastest per-op) takes the last two so the final
# tile's cast is short
CAST_ENG = {}
for _i in range(NT - 2):
    CAST_ENG[_i] = "scalar" if _i % 2 == 0 else "gpsimd"
CAST_ENG[NT - 2] = "vector"
CAST_ENG[NT - 1] = "vector"


def emit_kernel(ctx, tc, probs_d, targ_d, out_d):
    nc = tc.nc
    Alu = mybir.AluOpType
    from concourse.masks import make_identity

    consts = ctx.enter_context(tc.tile_pool(name="consts", bufs=1))
    probs_pool = ctx.enter_context(tc.tile_pool(name="probs_pool", bufs=NT))
    cast_pool = ctx.enter_context(tc.tile_pool(name="cast_pool", bufs=NT))
    eq_pool = ctx.enter_context(tc.tile_pool(name="eq_pool", bufs=NT))
    psum = ctx.enter_context(tc.tile_pool(name="psum", bufs=1, space="PSUM"))

    # 1) all probs tile loads first: HWDGE on the SP ring, plain fp32.
    # SP issues ~0.7us per DMA; the SDMA engines stream ~1.4us per tile, so
    # descriptor generation stays ahead of the data.
    pf32s = []
    for i in range(NT):
        pf32 = probs_pool.tile([P, C], F32, tag="pf32", name=f"pf32_{i}")
        nc.sync.dma_start(out=pf32[:], in_=probs_d[i * P:(i + 1) * P, :])
        pf32s.append(pf32)

    # 2) targets: one [16, 128] contiguous load on the OTHER HWDGE ring
    # (ACT) so it lands early while SP is busy issuing probs loads. Then
    # PE-transpose to [128, 16] so tile i's targets sit at column i as
    # per-partition scalars.
    t_rows_i32 = consts.tile([NT, P], I32, tag="t_rows_i32")
    nc.scalar.dma_start(out=t_rows_i32[:], in_=targ_d.rearrange("(i p) -> i p", p=P))

    # 3) constants
    ones = consts.tile([P, 1], F16, tag="ones")
    nc.vector.memset(ones[:], 1.0)
    iota_i16 = consts.tile([P, C], I16, tag="iota_i16")
    nc.gpsimd.iota(iota_i16[:], pattern=[[1, C]], base=0, channel_multiplier=0)
    iota_f16 = consts.tile([P, C], F16, tag="iota_f16")
    nc.vector.tensor_copy(iota_f16[:], iota_i16[:])
    identity = consts.tile([P, P], F32, tag="identity")
    make_identity(nc, identity[:])

    t_rows_f32 = consts.tile([NT, P], F32, tag="t_rows_f32")
    # gpsimd (not DVE) so the PE transpose below has single-engine producers
    nc.gpsimd.tensor_copy(t_rows_f32[:], t_rows_i32[:])
    t_ps = psum.tile([P, NT], F32, tag="t_ps")
    nc.tensor.transpose(t_ps[:], t_rows_f32[:], identity[:NT, :NT])
    t_cols = consts.tile([P, NT], F32, tag="t_cols")
    nc.vector.tensor_copy(t_cols[:], t_ps[:])
    t_cols_i32 = consts.tile([P, NT], I32, tag="t_cols_i32")
    nc.vector.tensor_copy(t_cols_i32[:], t_ps[:])

    # pt[p, j] = probs[128j + p, t] in ONE indirect gather (exact fp32);
    # its 2048 descriptors ride the (otherwise idle) SWDGE ring, which is
    # 8x the default size.
    rows_i32 = consts.tile([P, NT], I32, tag="rows_i32")
    nc.gpsimd.iota(rows_i32[:], pattern=[[P, NT]], base=0, channel_multiplier=1)
    offs = consts.tile([P, NT], I32, tag="offs")
    nc.vector.tensor_scalar(out=offs[:], in0=rows_i32[:], scalar1=float(C),
                            scalar2=None, op0=Alu.mult)
    nc.vector.tensor_tensor(out=offs[:], in0=offs[:], in1=t_cols_i32[:],
                            op=Alu.add)
    pt_all = consts.tile([P, NT], F32, tag="pt_all")
    nc.gpsimd.indirect_dma_start(
        out=pt_all[:], out_offset=None,
        in_=probs_d.rearrange("a b -> (a b)")[:, None],
        in_offset=bass.IndirectOffsetOnAxis(ap=offs[:], axis=0),
    )

    # persistent accumulators
    cs_ps = [psum.tile([1, CH], F32, tag=f"cs_ps{h}", name=f"cs_ps{h}")
             for h in range(2)]
    hs_ps = [psum.tile([1, CH], F32, tag=f"hs_ps{h}", name=f"hs_ps{h}")
             for h in range(2)]

    # 4a) one-hot rows — they depend only on iota/t_cols, so DVE builds
    # them while the probs DMAs stream in.
    eqs = []
    for i in range(NT):
        eq = eq_pool.tile([P, C], F16, tag="eq", name=f"eq_{i}")
        nc.vector.tensor_scalar(
            out=eq[:], in0=iota_f16[:], scalar1=t_cols[:, i:i + 1], scalar2=None,
            op0=Alu.is_equal,
        )
        eqs.append(eq)

    # 4b) focal staging FIRST in ACT program order (pt_all lands ~13us;
    # if this trailed the casts in ACT's FIFO, the PE colsum block would
    # stall behind the focal transpose until the second-to-last cast):
    # ACT stages [pt | ln(pt)] side by side so the DVE reduce depends on a
    # single engine.
    pl = consts.tile([P, 2 * NT], F32, tag="pl")
    nc.scalar.copy(pl[:, 0:NT], pt_all[:])
    nc.scalar.activation(pl[:, NT:2 * NT], pt_all[:],
                         mybir.ActivationFunctionType.Ln)

    # 4c) DVE focal reduce: focal[p] = sum_j (pt - 1) * ln(pt). Must
    # precede the DVE casts in program order — the PE colsum block sits
    # behind the focal transpose, which waits on this.
    junk = consts.tile([P, NT], F32, tag="junk")
    focal = consts.tile([P, 1], F32, tag="focal")
    nc.vector.scalar_tensor_tensor(
        out=junk[:], in0=pl[:, 0:NT], scalar=1.0, in1=pl[:, NT:2 * NT],
        op0=Alu.subtract, op1=Alu.mult, accum_out=focal[:],
    )

    # 4d) fp16 casts of the probs tiles, DMA-paced, spread across ACT /
    # GpSimd / DVE per CAST_ENG.
    pf16s = []
    for i in range(NT):
        pf16 = cast_pool.tile([P, C], F16, tag="pf16", name=f"pf16_{i}")
        eng = CAST_ENG[i]
        if eng == "scalar":
            nc.scalar.copy(pf16[:], pf32s[i][:])
        elif eng == "gpsimd":
            nc.gpsimd.tensor_copy(pf16[:], pf32s[i][:])
        else:
            nc.vector.tensor_copy(pf16[:], pf32s[i][:])
        pf16s.append(pf16)

    # 6a) all histogram matmuls as one dense block ahead of the colsum
    # matmuls in PE program order: ~7us of early PE work warms the HAM
    # clock gate before the DMA-paced colsum matmuls arrive.
    for i in range(NT):
        first, last = (i == 0), (i == NT - 1)
        for h in range(2):
            sl = slice(h * CH, (h + 1) * CH)
            nc.tensor.matmul(hs_ps[h][:], ones[:], eqs[i][:, sl],
                             start=first, stop=last)

    # 6b) focal partition-reduce on PE between the hist and colsum blocks
    # (focal is ready mid-stream; it must not trail the colsum matmuls).
    fc_t = psum.tile([1, P], F32, tag="fc_t")
    nc.tensor.transpose(fc_t[:], focal[:], identity[:])

    # 6c) cast-paced colsum matmuls.
    for i in range(NT):
        first, last = (i == 0), (i == NT - 1)
        for h in range(2):
            sl = slice(h * CH, (h + 1) * CH)
            nc.tensor.matmul(cs_ps[h][:], ones[:], pf16s[i][:, sl],
                             start=first, stop=last)

    # 7) pack [colsum | hist | focal_sum] into one row, single output DMA.
    # hist halves + focal drain on ACT mid-stream; the colsum halves drain
    # in parallel (DVE half 0, ACT half 1) right after the last matmul.
    out_sb = consts.tile([1, OUT_W], F32, tag="out_sb")
    for h in range(2):
        nc.scalar.copy(out_sb[:, C + h * CH:C + (h + 1) * CH], hs_ps[h][:])
    fc_row = consts.tile([1, P], F32, tag="fc_row")
    nc.scalar.activation(fc_row[:], fc_t[:],
                         mybir.ActivationFunctionType.Copy,
                         accum_out=out_sb[:, 2 * C:2 * C + 1])
    nc.vector.tensor_copy(out_sb[:, 0:CH], cs_ps[0][:])
    nc.scalar.copy(out_sb[:, CH:2 * CH], cs_ps[1][:])
    nc.sync.dma_start(out=out_d[:, :], in_=out_sb[:])


def _split_multi_waits(nc):
    """The walrus build in this env encodes at most ONE sync wait per
    instruction (newer Tile emits several, e.g. on its tail drain). Hoist
    extra waits onto EventSemaphore carrier instructions inserted just
    before, on the same engine — same-engine program order makes this
    semantically identical."""
    n = 0
    for f in nc.m.functions:
        for blk in f.blocks:
            il = blk.instructions
            i = 0
            while i < len(il):
                inst = il[i]
                si = inst.sync_info
                ws = list(si.on_wait) if si is not None else []
                if len(ws) > 1:
                    for w in ws[:-1]:
                        ev = mybir.InstEventSemaphore(
                            name=f"I-waitsplit-{n}", ins=[], outs=[])
                        n += 1
                        ev.engine = inst.engine
                        ev.sync_info = mybir.SyncInfo(on_wait=[w], on_update=[])
                        il.insert(i, ev)
                        i += 1
                    inst.sync_info = mybir.SyncInfo(
                        on_wait=[ws[-1]], on_update=list(si.on_update))
                i += 1


def _compact_sem_ids(nc, base=3):
    """Tile/bass allocate semaphore ids from ~151 up, but the walrus codegen
    epilogue clears every id from 2 to its allocation cap — one instruction
    each. Densely remap every semaphore this program touches down to
    [base, base+n) so --max-sem-num can be tiny and the clear loop near-empty.
    ids 0-2 are left for the compiler's own preamble/epilogue barriers."""
    def insts():
        for f in nc.m.functions:
            for b in f.blocks:
                yield from b.instructions

    used = set()
    for inst in insts():
        si = inst.sync_info
        if si:
            for w in list(si.on_wait):
                if w.sync_type == "semaphore":
                    used.add(w.id)
            for u in list(si.on_update):
                if u.sync_type == "semaphore":
                    used.add(u.id)
    m = {old: base + i for i, old in enumerate(sorted(used))}
    for inst in insts():
        si = inst.sync_info
        if si:
            ws, us = list(si.on_wait), list(si.on_update)
            changed = False
            for w in ws:
                if w.sync_type == "semaphore" and w.id in m:
                    w.id = m[w.id]
                    changed = True
            for u in us:
                if u.sync_type == "semaphore" and u.id in m:
                    u.id = m[u.id]
                    changed = True
            if changed:
                inst.sync_info = mybir.SyncInfo(on_wait=ws, on_update=us)
        if (type(inst).__name__ == "InstISA"
                and getattr(inst, "op_name", "") == "EVENT_SEMAPHORE_RANGE_CLEAR"):
            d = inst.ant_dict
            ids = [m[x] for x in range(d["range_first"], d["range_last"] + 1)
                   if x in m]
            nf, nl = (min(ids), max(ids)) if ids else (base, base)
            d["range_first"], d["range_last"] = nf, nl
            v = list(inst.instr)
            v[13], v[14] = nf, nl
            inst.instr = v
            inst.ant_dict = d


_cached_nc = {}


def build_nc(split_waits=True):
    global _cached_nc
    if split_waits in _cached_nc:
        return _cached_nc[split_waits]
    from contextlib import ExitStack

    # 64 KiB SWDGE ring: half the baseline's — the pt gather's 2048
    # descriptors wrap once (Q7 briefly blocks mid-stream, off the tail),
    # and the freed 64 KiB/partition makes room for the fp32 tile pool.
    nc = bass.Bass("TRN2", dynamic_dma_scratch_size=65536)
    probs_d = nc.dram_tensor("probs", [BC, C], F32, kind="ExternalInput").ap()
    targ_d = nc.dram_tensor("targets", [BC], I32, kind="ExternalInput").ap()
    out_d = nc.dram_tensor("out_all", [1, OUT_W], F32, kind="ExternalOutput").ap()

    with tile.TileContext(nc) as tc:
        with ExitStack() as ctx:
            emit_kernel(ctx, tc, probs_d, targ_d, out_d)
    if split_waits:
        _split_multi_waits(nc)
    _compact_sem_ids(nc)
    _cached_nc[split_waits] = nc
    return nc


def make_in_maps(probs, targets):
    probs = np.ascontiguousarray(np.asarray(probs), dtype=np.float32)
    targets = np.asarray(targets).astype(np.int32)
    assert probs.shape == (B, C) and targets.shape == (B,)
    return [
        {
            "probs": probs[k * BC:(k + 1) * BC],
            "targets": np.ascontiguousarray(targets[k * BC:(k + 1) * BC]),
        }
        for k in range(NCORES)
    ]


def combine(results):
    cs = np.zeros(C, np.float64)
    hs = np.zeros(C, np.float64)
    fc = 0.0
    for r in results:
        row = r["out_all"].reshape(OUT_W).astype(np.float64)
        cs += row[0:C]
        hs += row[C:2 * C]
        fc += row[2 * C]
    loss_cls = fc / B
    loss_cal = float(np.mean(np.abs(cs / B - hs / B)))
    return np.asarray(loss_cls + 1.0 * loss_cal, dtype=np.float32)


def run_spmd(probs, targets, **kwargs):
    nc = build_nc()
    in_maps = make_in_maps(probs, targets)
    return run_bass_kernel_spmd(nc, in_maps, list(range(NCORES)), **kwargs)


def kernel(probs, targets):
    res = run_spmd(probs, targets)
    return combine(res.results)


# revision 7
# speedup vs baseline: 1.2125x; 1.2125x over previous
"""FocalLoss + MDCA loss kernel for TRN2, 8-core data-parallel. v3.

reference:
    loss_cls = mean_i[-(1-pt_i) * log(pt_i)],  pt_i = probs[i, targets[i]]
    loss_cal = mean_c |mean_i probs[i,c] - count_c/B|
    out = loss_cls + loss_cal        (GAMMA=1, BETA=1)

Strategy: shard batch (16384) across 8 cores (2048 rows each). Each core:
  - streams its probs shard HBM->SBUF with an inline fp32->fp16 cast (SWDGE)
    into dedicated per-tile buffers (no slot-reuse waits on the DMAs). The
    conversion rides the DMA datapath for free — a compute-engine cast of
    the same bytes costs 13-25us (measured), so SWDGE-convert wins even
    with its Q7 descriptor-generation cost.
  - Q7 emission order is arranged so descriptor generation never starves
    the SDMA ring: 8 tile-DMAs up front (~11us of queued stream work),
    then the iota/identity/gather const block, then the remaining 8.
  - PE matmul ones[128,1]^T @ probs_fp16 accumulates column sums in PSUM
  - DVE builds one-hot rows eq[p,c] = (c == target_p) from an iota
    constant, PE matmul ones^T @ eq accumulates the target histogram
  - pt[p,j] = probs[128j + p, t] in ONE indirect SWDGE gather (exact fp32);
    the focal chain (ACT [pt|ln pt], DVE fused (pt-1)*ln(pt) row-sum, PE
    transpose, ACT accumulate) completes before the stream tail
  - tail: last tile's 2 colsum matmuls -> PSUM drains split DVE/ACT in
    parallel -> one [1,2001] f32 output DMA
Host combines the 8 cores' colsum/hist/focal partials into the scalar loss
(the gather/unshard step).

The walrus build in this env encodes at most ONE sync wait per instruction;
_split_multi_waits post-processes the scheduled program to hoist extra waits
onto same-engine EventSemaphore carriers.

The walrus codegen epilogue clears every semaphore id from 2 up to its
allocation cap (default 256) with one EVENT_SEMAPHORE each, split across
engines — ~6us of cleanup inside the profiled window. _compact_sem_ids
densely remaps the ~15 semaphores this program touches down to ids 3..17
so --max-sem-num=32 shrinks that loop to a few hundred ns.
"""

import numpy as np

import concourse.bass as bass
import concourse.bass_utils as _bu
import concourse.mybir as mybir
import concourse.tile as tile
from concourse.bass_utils import run_bass_kernel_spmd

if not getattr(_bu.bir_verify_and_optimise, "_sem_capped", False):
    _orig_bvo = _bu.bir_verify_and_optimise

    def _bvo_capped(*args, **kwargs):
        import concourse.bass_utils as bu

        orig_run = bu.run_command

        def run_with_cap(cmd, **kw):
            if any("codegen" in str(c) for c in cmd):
                cmd = list(cmd) + ["--max-sem-num=32"]
            return orig_run(cmd, **kw)

        bu.run_command = run_with_cap
        try:
            return _orig_bvo(*args, **kwargs)
        finally:
            bu.run_command = orig_run

    _bvo_capped._sem_capped = True
    _bu.bir_verify_and_optimise = _bvo_capped

B, C = 16384, 1000
NCORES = 8
BC = B // NCORES  # 2048 rows per core
P = 128
NT = BC // P      # 16 batch tiles per core
CH = 500          # matmul half free-dim (PSUM bank = 512 fp32)
OUT_W = 2001      # [colsum 0:1000 | hist 1000:2000 | focal_sum 2000]
NFRONT = 8        # tile-DMAs emitted before the Q7 const block

F32 = mybir.dt.float32
F16 = mybir.dt.float16
I16 = mybir.dt.int16
I32 = mybir.dt.int32


def emit_kernel(ctx, tc, probs_d, targ_d, out_d):
    nc = tc.nc
    Alu = mybir.AluOpType
    from concourse.masks import make_identity

    consts = ctx.enter_context(tc.tile_pool(name="consts", bufs=1))
    probs_pool = ctx.enter_context(tc.tile_pool(name="probs_pool", bufs=NT))
    eq_pool = ctx.enter_context(tc.tile_pool(name="eq_pool", bufs=NT))
    psum = ctx.enter_context(tc.tile_pool(name="psum", bufs=1, space="PSUM"))

    # 1) the first NFRONT probs tile loads go out back-to-back: ~0.66us of
    # Q7 descriptor generation each, giving the SDMA engines ~11us of
    # queued stream work before Q7 turns to the const block below. The
    # SDMA ring therefore never starves while iota/identity/gather emit.
    def load_tile(i):
        pf16 = probs_pool.tile([P, C], F16, tag="pf16", name=f"pf16_{i}")
        nc.gpsimd.dma_start(out=pf16[:], in_=probs_d[i * P:(i + 1) * P, :])
        return pf16

    pf16s = [load_tile(i) for i in range(NFRONT)]

    # 2) targets: one [16, 128] contiguous load (HWDGE), PE-transpose to
    # [128, 16] so tile i's targets sit at column i as per-partition scalars.
    t_rows_i32 = consts.tile([NT, P], I32, tag="t_rows_i32")
    nc.sync.dma_start(out=t_rows_i32[:], in_=targ_d.rearrange("(i p) -> i p", p=P))

    # 3) constants
    ones = consts.tile([P, 1], F16, tag="ones")
    nc.vector.memset(ones[:], 1.0)
    iota_i16 = consts.tile([P, C], I16, tag="iota_i16")
    nc.gpsimd.iota(iota_i16[:], pattern=[[1, C]], base=0, channel_multiplier=0)
    iota_f16 = consts.tile([P, C], F16, tag="iota_f16")
    nc.vector.tensor_copy(iota_f16[:], iota_i16[:])
    identity = consts.tile([P, P], F32, tag="identity")
    make_identity(nc, identity[:])

    t_rows_f32 = consts.tile([NT, P], F32, tag="t_rows_f32")
    # gpsimd (not DVE) so the PE transpose below has single-engine producers
    nc.gpsimd.tensor_copy(t_rows_f32[:], t_rows_i32[:])
    t_ps = psum.tile([P, NT], F32, tag="t_ps")
    nc.tensor.transpose(t_ps[:], t_rows_f32[:], identity[:NT, :NT])
    t_cols = consts.tile([P, NT], F32, tag="t_cols")
    nc.vector.tensor_copy(t_cols[:], t_ps[:])
    t_cols_i32 = consts.tile([P, NT], I32, tag="t_cols_i32")
    nc.vector.tensor_copy(t_cols_i32[:], t_ps[:])

    # pt[p, j] = probs[128j + p, t] in ONE indirect gather (exact fp32).
    # Emitted mid-way through the probs loads: its data lands behind tile
    # NFRONT-1's, so the focal chain completes well before the stream tail.
    rows_i32 = consts.tile([P, NT], I32, tag="rows_i32")
    nc.gpsimd.iota(rows_i32[:], pattern=[[P, NT]], base=0, channel_multiplier=1)
    offs = consts.tile([P, NT], I32, tag="offs")
    nc.vector.tensor_scalar(out=offs[:], in0=rows_i32[:], scalar1=float(C),
                            scalar2=None, op0=Alu.mult)
    nc.vector.tensor_tensor(out=offs[:], in0=offs[:], in1=t_cols_i32[:],
                            op=Alu.add)
    pt_all = consts.tile([P, NT], F32, tag="pt_all")
    nc.gpsimd.indirect_dma_start(
        out=pt_all[:], out_offset=None,
        in_=probs_d.rearrange("a b -> (a b)")[:, None],
        in_offset=bass.IndirectOffsetOnAxis(ap=offs[:], axis=0),
    )

    # remaining probs loads: Q7 emits these by ~23us; the SDMA engines are
    # still draining tiles 0..NFRONT-1 until ~19us, so the stream is gapless.
    pf16s += [load_tile(i) for i in range(NFRONT, NT)]

    # persistent accumulators
    cs_ps = [psum.tile([1, CH], F32, tag=f"cs_ps{h}", name=f"cs_ps{h}")
             for h in range(2)]
    hs_ps = [psum.tile([1, CH], F32, tag=f"hs_ps{h}", name=f"hs_ps{h}")
             for h in range(2)]

    # 4a) all one-hot rows first — they depend only on iota/t_cols, so DVE
    # builds them while the probs DMAs stream in.
    eqs = []
    for i in range(NT):
        eq = eq_pool.tile([P, C], F16, tag="eq", name=f"eq_{i}")
        nc.vector.tensor_scalar(
            out=eq[:], in0=iota_f16[:], scalar1=t_cols[:, i:i + 1], scalar2=None,
            op0=Alu.is_equal,
        )
        eqs.append(eq)

    # 4b) all histogram matmuls as one dense DMA-independent block: early
    # back-to-back PE work warms the HAM clock gate (2.4 GHz) before the
    # DMA-paced colsum matmuls arrive.
    for i in range(NT):
        first, last = (i == 0), (i == NT - 1)
        for h in range(2):
            sl = slice(h * CH, (h + 1) * CH)
            nc.tensor.matmul(hs_ps[h][:], ones[:], eqs[i][:, sl],
                             start=first, stop=last)

    # 4c) DMA-paced colsum matmuls.
    for i in range(NT):
        pf16 = pf16s[i]
        first, last = (i == 0), (i == NT - 1)
        for h in range(2):
            sl = slice(h * CH, (h + 1) * CH)
            nc.tensor.matmul(cs_ps[h][:], ones[:], pf16[:, sl],
                             start=first, stop=last)

    # 5) focal tail: focal[p] = sum_i (pt - 1) * ln(pt).
    # Stage [pt | ln(pt)] side by side via ACT so the DVE reduce depends on a
    # single engine.
    pl = consts.tile([P, 2 * NT], F32, tag="pl")
    nc.scalar.copy(pl[:, 0:NT], pt_all[:])
    nc.scalar.activation(pl[:, NT:2 * NT], pt_all[:],
                         mybir.ActivationFunctionType.Ln)
    junk = consts.tile([P, NT], F32, tag="junk")
    focal = consts.tile([P, 1], F32, tag="focal")
    nc.vector.scalar_tensor_tensor(
        out=junk[:], in0=pl[:, 0:NT], scalar=1.0, in1=pl[:, NT:2 * NT],
        op0=Alu.subtract, op1=Alu.mult, accum_out=focal[:],
    )
    # reduce focal over partitions: PE transpose to a row, ACT accumulates
    fc_t = psum.tile([1, P], F32, tag="fc_t")
    nc.tensor.transpose(fc_t[:], focal[:], identity[:])

    # 6) pack [colsum | hist | focal_sum] into one row, single output DMA.
    # hist halves drain on ACT mid-stream; the colsum halves drain in
    # parallel right after the last matmul (DVE half 0, ACT half 1).
    out_sb = consts.tile([1, OUT_W], F32, tag="out_sb")
    for h in range(2):
        nc.scalar.copy(out_sb[:, C + h * CH:C + (h + 1) * CH], hs_ps[h][:])
    fc_row = consts.tile([1, P], F32, tag="fc_row")
    nc.scalar.activation(fc_row[:], fc_t[:],
                         mybir.ActivationFunctionType.Copy,
                         accum_out=out_sb[:, 2 * C:2 * C + 1])
    nc.vector.tensor_copy(out_sb[:, 0:CH], cs_ps[0][:])
    nc.scalar.copy(out_sb[:, CH:2 * CH], cs_ps[1][:])
    nc.sync.dma_start(out=out_d[:, :], in_=out_sb[:])


def _split_multi_waits(nc):
    """The walrus build in this env encodes at most ONE sync wait per
    instruction (newer Tile emits several, e.g. on its tail drain). Hoist
    extra waits onto EventSemaphore carrier instructions inserted just
    before, on the same engine — same-engine program order makes this
    semantically identical."""
    n = 0
    for f in nc.m.functions:
        for blk in f.blocks:
            il = blk.instructions
            i = 0
            while i < len(il):
                inst = il[i]
                si = inst.sync_info
                ws = list(si.on_wait) if si is not None else []
                if len(ws) > 1:
                    for w in ws[:-1]:
                        ev = mybir.InstEventSemaphore(
                            name=f"I-waitsplit-{n}", ins=[], outs=[])
                        n += 1
                        ev.engine = inst.engine
                        ev.sync_info = mybir.SyncInfo(on_wait=[w], on_update=[])
                        il.insert(i, ev)
                        i += 1
                    inst.sync_info = mybir.SyncInfo(
                        on_wait=[ws[-1]], on_update=list(si.on_update))
                i += 1


def _compact_sem_ids(nc, base=3):
    """Tile/bass allocate semaphore ids from ~151 up, but the walrus codegen
    epilogue clears every id from 2 to its allocation cap — one instruction
    each. Densely remap every semaphore this program touches down to
    [base, base+n) so --max-sem-num can be tiny and the clear loop
    near-empty. ids 0-2 are left for the compiler's own barriers."""
    def insts():
        for f in nc.m.functions:
            for b in f.blocks:
                yield from b.instructions

    used = set()
    for inst in insts():
        si = inst.sync_info
        if si:
            for w in list(si.on_wait):
                if w.sync_type == "semaphore":
                    used.add(w.id)
            for u in list(si.on_update):
                if u.sync_type == "semaphore":
                    used.add(u.id)
    m = {old: base + i for i, old in enumerate(sorted(used))}
    for inst in insts():
        si = inst.sync_info
        if si:
            ws, us = list(si.on_wait), list(si.on_update)
            changed = False
            for w in ws:
                if w.sync_type == "semaphore" and w.id in m:
                    w.id = m[w.id]
                    changed = True
            for u in us:
                if u.sync_type == "semaphore" and u.id in m:
                    u.id = m[u.id]
                    changed = True
            if changed:
                inst.sync_info = mybir.SyncInfo(on_wait=ws, on_update=us)
        if (type(inst).__name__ == "InstISA"
                and getattr(inst, "op_name", "") == "EVENT_SEMAPHORE_RANGE_CLEAR"):
            d = inst.ant_dict
            ids = [m[x] for x in range(d["range_first"], d["range_last"] + 1)
                   if x in m]
            nf, nl = (min(ids), max(ids)) if ids else (base, base)
            d["range_first"], d["range_last"] = nf, nl
            v = list(inst.instr)
            v[13], v[14] = nf, nl
            inst.instr = v
            inst.ant_dict = d


_cached_nc = {}


def build_nc(split_waits=True):
    global _cached_nc
    if split_waits in _cached_nc:
        return _cached_nc[split_waits]
    from contextlib import ExitStack

    nc = bass.Bass("TRN2", dynamic_dma_scratch_size=131072)
    probs_d = nc.dram_tensor("probs", [BC, C], F32, kind="ExternalInput").ap()
    targ_d = nc.dram_tensor("targets", [BC], I32, kind="ExternalInput").ap()
    out_d = nc.dram_tensor("out_all", [1, OUT_W], F32, kind="ExternalOutput").ap()

    with tile.TileContext(nc) as tc:
        with ExitStack() as ctx:
            emit_kernel(ctx, tc, probs_d, targ_d, out_d)
    if split_waits:
        _split_multi_waits(nc)
    _compact_sem_ids(nc)
    _cached_nc[split_waits] = nc
    return nc


def make_in_maps(probs, targets):
    probs = np.ascontiguousarray(np.asarray(probs), dtype=np.float32)
    targets = np.asarray(targets).astype(np.int32)
    assert probs.shape == (B, C) and targets.shape == (B,)
    return [
        {
            "probs": probs[k * BC:(k + 1) * BC],
            "targets": np.ascontiguousarray(targets[k * BC:(k + 1) * BC]),
        }
        for k in range(NCORES)
    ]


def combine(results):
    cs = np.zeros(C, np.float64)
    hs = np.zeros(C, np.float64)
    fc = 0.0
    for r in results:
        row = r["out_all"].reshape(OUT_W).astype(np.float64)
        cs += row[0:C]
        hs += row[C:2 * C]
        fc += row[2 * C]
    loss_cls = fc / B
    loss_cal = float(np.mean(np.abs(cs / B - hs / B)))
    return np.asarray(loss_cls + 1.0 * loss_cal, dtype=np.float32)


def run_spmd(probs, targets, **kwargs):
    nc = build_nc()
    in_maps = make_in_maps(probs, targets)
    return run_bass_kernel_spmd(nc, in_maps, list(range(NCORES)), **kwargs)


def kernel(probs, targets):
    res = run_spmd(probs, targets)
    return combine(res.results)


# revision 13
# speedup vs baseline: 1.5580x; 1.2850x over previous
"""FocalLoss + MDCA loss kernel for TRN2, 8-core data-parallel. v4.

reference:
    loss_cls = mean_i[-(1-pt_i) * log(pt_i)],  pt_i = probs[i, targets[i]]
    loss_cal = mean_c |mean_i probs[i,c] - count_c/B|
    out = loss_cls + loss_cal        (GAMMA=1, BETA=1)

Strategy: shard batch (16384) across 8 cores (2048 rows each). Each core:
  - streams its probs shard HBM->SBUF with an inline fp32->fp16 cast (SWDGE)
    as EIGHT [128, 2000] big-tiles: big-tile k covers rows 256k..256k+255
    with partition p holding rows (256k+2p, 256k+2p+1) side by side. Read
    descriptors are 8000 B contiguous (vs 4000 B for one-row tiles): half
    the descriptor count, half the per-packet overhead, and half the load
    on the slow SWDGE engines 7/15.
  - Q7 emission order keeps the baseline's proven shape: 2 big-tile DMAs
    up front, then the const block + the ONE indirect pt gather (its 2048
    tiny descriptors drain while the ring is still shallow), then the
    remaining 6 big-tile DMAs.
  - PE matmul ones[128,1]^T @ probs_fp16 accumulates column sums in PSUM
    (4 x [128,500] per big-tile, same 32 total as before)
  - DVE builds one-hot rows eq[p, j*1000+c] = (c == targets[256k+2p+j]);
    PE matmul ones^T @ eq accumulates the target histogram (exact)
  - pt[p, kj] = probs[256k+2p+j, t] via the indirect gather (exact fp32);
    the focal chain (ACT [pt|ln pt], DVE fused (pt-1)*ln(pt) row-sum, PE
    transpose, ACT accumulate) completes mid-stream
  - tail: last big-tile's 4 colsum matmuls -> PSUM drains split DVE/ACT in
    parallel -> one [1,2001] f32 output DMA
Host combines the 8 cores' colsum/hist/focal partials into the scalar loss
(the gather/unshard step).

The walrus build in this env encodes at most ONE sync wait per instruction;
_split_multi_waits post-processes the scheduled program to hoist extra waits
onto same-engine EventSemaphore carriers.

_compact_sem_ids densely remaps the ~15 semaphores this program touches down
to ids 3..18 and --max-sem-num caps the allocator. (The runtime's end-of-NEFF
sweep still clears all 256 ids — measured fixed cost — but the compact ids
keep the program itself well inside any cap.)
"""

import numpy as np

import concourse.bass as bass
import concourse.bass_utils as _bu
import concourse.mybir as mybir
import concourse.tile as tile
from concourse.bass_utils import run_bass_kernel_spmd

if not getattr(_bu.bir_verify_and_optimise, "_sem_capped", False):
    _orig_bvo = _bu.bir_verify_and_optimise

    def _bvo_capped(*args, **kwargs):
        import concourse.bass_utils as bu

        orig_run = bu.run_command

        def run_with_cap(cmd, **kw):
            if any("codegen" in str(c) for c in cmd):
                cmd = list(cmd) + ["--max-sem-num=32"]
            return orig_run(cmd, **kw)

        bu.run_command = run_with_cap
        try:
            return _orig_bvo(*args, **kwargs)
        finally:
            bu.run_command = orig_run

    _bvo_capped._sem_capped = True
    _bu.bir_verify_and_optimise = _bvo_capped

B, C = 16384, 1000
NCORES = 8
BC = B // NCORES  # 2048 rows per core
P = 128
NB = 8            # big-tiles per core: [128, 2000], 256 rows each
J = 2             # rows per partition per big-tile
W = J * C         # 2000 fp16 columns per big-tile
NT = BC // P      # 16 logical 128-row groups (for targets/pt layout)
CH = 500          # matmul half free-dim (PSUM bank = 512 fp32)
OUT_W = 2001      # [colsum 0:1000 | hist 1000:2000 | focal_sum 2000]
NFRONT = 2        # big-tile DMAs emitted before the Q7 const/gather block

F32 = mybir.dt.float32
F16 = mybir.dt.float16
I16 = mybir.dt.int16
I32 = mybir.dt.int32


def emit_kernel(ctx, tc, probs_d, targ_d, out_d):
    nc = tc.nc
    Alu = mybir.AluOpType
    from concourse.masks import make_identity

    consts = ctx.enter_context(tc.tile_pool(name="consts", bufs=1))
    probs_pool = ctx.enter_context(tc.tile_pool(name="probs_pool", bufs=NB))
    eq_pool = ctx.enter_context(tc.tile_pool(name="eq_pool", bufs=NT))
    psum = ctx.enter_context(tc.tile_pool(name="psum", bufs=1, space="PSUM"))

    # 1) first two big-tile loads start immediately (SDMA drains them while
    # Q7 builds the constants below); partition p of big-tile k reads DRAM
    # rows 256k+2p, 256k+2p+1 — one contiguous 8000 B descriptor.
    def load_tile(k):
        pf16 = probs_pool.tile([P, W], F16, tag="pf16", name=f"pf16_{k}")
        nc.gpsimd.dma_start(
            out=pf16[:],
            in_=probs_d[k * J * P:(k + 1) * J * P, :].rearrange(
                "(p j) c -> p (j c)", p=P, j=J),
        )
        return pf16

    pf16s = [load_tile(k) for k in range(NFRONT)]

    # 2) targets: one [16, 128] contiguous load (HWDGE), PE-transpose to
    # [128, 16] so column i holds targets[128i+p] as per-partition scalars.
    # NOTE: the hist/eq/pt logic below keeps this BASELINE row grouping —
    # the histogram is a multiset count and focal a plain sum, so they
    # don't need to match the big-tile row->partition interleave; only the
    # colsum matmul slices track the new probs layout.
    t_rows_i32 = consts.tile([NT, P], I32, tag="t_rows_i32")
    nc.sync.dma_start(out=t_rows_i32[:], in_=targ_d.rearrange("(i p) -> i p", p=P))

    # 3) constants
    ones = consts.tile([P, 1], F16, tag="ones")
    nc.vector.memset(ones[:], 1.0)
    iota_i16 = consts.tile([P, C], I16, tag="iota_i16")
    nc.gpsimd.iota(iota_i16[:], pattern=[[1, C]], base=0, channel_multiplier=0)
    iota_f16 = consts.tile([P, C], F16, tag="iota_f16")
    nc.vector.tensor_copy(iota_f16[:], iota_i16[:])
    identity = consts.tile([P, P], F32, tag="identity")
    make_identity(nc, identity[:])

    t_rows_f32 = consts.tile([NT, P], F32, tag="t_rows_f32")
    # gpsimd (not DVE) so the PE transpose below has single-engine producers
    nc.gpsimd.tensor_copy(t_rows_f32[:], t_rows_i32[:])
    t_ps = psum.tile([P, NT], F32, tag="t_ps")
    nc.tensor.transpose(t_ps[:], t_rows_f32[:], identity[:NT, :NT])
    t_cols = consts.tile([P, NT], F32, tag="t_cols")
    nc.vector.tensor_copy(t_cols[:], t_ps[:])
    t_cols_i32 = consts.tile([P, NT], I32, tag="t_cols_i32")
    nc.vector.tensor_copy(t_cols_i32[:], t_ps[:])

    # pt[p, i] = probs[128i + p, t] in ONE indirect gather (exact fp32),
    # emitted before the bulk probs loads so its 2048 descriptors drain on a
    # near-quiet ring (the ring is 8x the default size).
    rows_i32 = consts.tile([P, NT], I32, tag="rows_i32")
    nc.gpsimd.iota(rows_i32[:], pattern=[[P, NT]], base=0, channel_multiplier=1)
    offs = consts.tile([P, NT], I32, tag="offs")
    nc.vector.tensor_scalar(out=offs[:], in0=rows_i32[:], scalar1=float(C),
                            scalar2=None, op0=Alu.mult)
    nc.vector.tensor_tensor(out=offs[:], in0=offs[:], in1=t_cols_i32[:],
                            op=Alu.add)
    pt_all = consts.tile([P, NT], F32, tag="pt_all")
    nc.gpsimd.indirect_dma_start(
        out=pt_all[:], out_offset=None,
        in_=probs_d.rearrange("a b -> (a b)")[:, None],
        in_offset=bass.IndirectOffsetOnAxis(ap=offs[:], axis=0),
    )

    # remaining big-tile loads
    pf16s += [load_tile(k) for k in range(NFRONT, NB)]

    # persistent accumulators
    cs_ps = [psum.tile([1, CH], F32, tag=f"cs_ps{h}", name=f"cs_ps{h}")
             for h in range(2)]
    hs_ps = [psum.tile([1, CH], F32, tag=f"hs_ps{h}", name=f"hs_ps{h}")
             for h in range(2)]

    # 4a) one-hot rows eq_i[p, c] = (c == targets[128i+p]) — baseline row
    # grouping, DVE-paced while the probs DMAs stream in.
    eqs = []
    for i in range(NT):
        eq = eq_pool.tile([P, C], F16, tag="eq", name=f"eq_{i}")
        nc.vector.tensor_scalar(
            out=eq[:], in0=iota_f16[:], scalar1=t_cols[:, i:i + 1], scalar2=None,
            op0=Alu.is_equal,
        )
        eqs.append(eq)

    # 4b) all histogram matmuls as one dense DMA-independent block: early
    # back-to-back PE work warms the HAM clock gate (2.4 GHz) before the
    # DMA-paced colsum matmuls arrive.
    for i in range(NT):
        first, last = (i == 0), (i == NT - 1)
        for h in range(2):
            sl = slice(h * CH, (h + 1) * CH)
            nc.tensor.matmul(hs_ps[h][:], ones[:], eqs[i][:, sl],
                             start=first, stop=last)

    # 4c) DMA-paced colsum matmuls: 4 x [128,500] per big-tile, banks
    # alternating so each bank accumulates 16 matmuls.
    for k in range(NB):
        for q in range(2 * J):
            sl = slice(q * CH, (q + 1) * CH)
            nc.tensor.matmul(cs_ps[q % 2][:], ones[:], pf16s[k][:, sl],
                             start=(k == 0 and q < 2),
                             stop=(k == NB - 1 and q >= 2 * J - 2))

    # 5) focal tail: focal[p] = sum_kj (pt - 1) * ln(pt).
    # Stage [pt | ln(pt)] side by side via ACT so the DVE reduce depends on a
    # single engine.
    pl = consts.tile([P, 2 * NT], F32, tag="pl")
    nc.scalar.copy(pl[:, 0:NT], pt_all[:])
    nc.scalar.activation(pl[:, NT:2 * NT], pt_all[:],
                         mybir.ActivationFunctionType.Ln)
    junk = consts.tile([P, NT], F32, tag="junk")
    focal = consts.tile([P, 1], F32, tag="focal")
    nc.vector.scalar_tensor_tensor(
        out=junk[:], in0=pl[:, 0:NT], scalar=1.0, in1=pl[:, NT:2 * NT],
        op0=Alu.subtract, op1=Alu.mult, accum_out=focal[:],
    )
    # reduce focal over partitions: PE transpose to a row, ACT accumulates
    fc_t = psum.tile([1, P], F32, tag="fc_t")
    nc.tensor.transpose(fc_t[:], focal[:], identity[:])

    # 6) pack [colsum | hist | focal_sum] into one row, single output DMA.
    # hist halves drain on ACT mid-stream; the colsum halves drain in
    # parallel right after the last matmul (DVE half 0, ACT half 1).
    out_sb = consts.tile([1, OUT_W], F32, tag="out_sb")
    for h in range(2):
        nc.scalar.copy(out_sb[:, C + h * CH:C + (h + 1) * CH], hs_ps[h][:])
    fc_row = consts.tile([1, P], F32, tag="fc_row")
    nc.scalar.activation(fc_row[:], fc_t[:],
                         mybir.ActivationFunctionType.Copy,
                         accum_out=out_sb[:, 2 * C:2 * C + 1])
    nc.vector.tensor_copy(out_sb[:, 0:CH], cs_ps[0][:])
    nc.scalar.copy(out_sb[:, CH:2 * CH], cs_ps[1][:])
    nc.sync.dma_start(out=out_d[:, :], in_=out_sb[:])


def _split_multi_waits(nc):
    """The walrus build in this env encodes at most ONE sync wait per
    instruction (newer Tile emits several, e.g. on its tail drain). Hoist
    extra waits onto EventSemaphore carrier instructions inserted just
    before, on the same engine — same-engine program order makes this
    semantically identical."""
    n = 0
    for f in nc.m.functions:
        for blk in f.blocks:
            il = blk.instructions
            i = 0
            while i < len(il):
                inst = il[i]
                si = inst.sync_info
                ws = list(si.on_wait) if si is not None else []
                if len(ws) > 1:
                    for w in ws[:-1]:
                        ev = mybir.InstEventSemaphore(
                            name=f"I-waitsplit-{n}", ins=[], outs=[])
                        n += 1
                        ev.engine = inst.engine
                        ev.sync_info = mybir.SyncInfo(on_wait=[w], on_update=[])
                        il.insert(i, ev)
                        i += 1
                    inst.sync_info = mybir.SyncInfo(
                        on_wait=[ws[-1]], on_update=list(si.on_update))
                i += 1


def _compact_sem_ids(nc, base=3):
    """Tile/bass allocate semaphore ids from ~151 up; remap every semaphore
    this program touches down to [base, base+n) so the program sits inside
    a small --max-sem-num cap. ids 0-2 stay free for the compiler's own
    barriers."""
    def insts():
        for f in nc.m.functions:
            for b in f.blocks:
                yield from b.instructions

    used = set()
    for inst in insts():
        si = inst.sync_info
        if si:
            for w in list(si.on_wait):
                if w.sync_type == "semaphore":
                    used.add(w.id)
            for u in list(si.on_update):
                if u.sync_type == "semaphore":
                    used.add(u.id)
    m = {old: base + i for i, old in enumerate(sorted(used))}
    for inst in insts():
        si = inst.sync_info
        if si:
            ws, us = list(si.on_wait), list(si.on_update)
            changed = False
            for w in ws:
                if w.sync_type == "semaphore" and w.id in m:
                    w.id = m[w.id]
                    changed = True
            for u in us:
                if u.sync_type == "semaphore" and u.id in m:
                    u.id = m[u.id]
                    changed = True
            if changed:
                inst.sync_info = mybir.SyncInfo(on_wait=ws, on_update=us)
        if (type(inst).__name__ == "InstISA"
                and getattr(inst, "op_name", "") == "EVENT_SEMAPHORE_RANGE_CLEAR"):
            d = inst.ant_dict
            ids = [m[x] for x in range(d["range_first"], d["range_last"] + 1)
                   if x in m]
            nf, nl = (min(ids), max(ids)) if ids else (base, base)
            d["range_first"], d["range_last"] = nf, nl
            v = list(inst.instr)
            v[13], v[14] = nf, nl
            inst.instr = v
            inst.ant_dict = d


_cached_nc = {}


def build_nc(split_waits=True):
    global _cached_nc
    if split_waits in _cached_nc:
        return _cached_nc[split_waits]
    from contextlib import ExitStack

    nc = bass.Bass("TRN2", dynamic_dma_scratch_size=131072)
    probs_d = nc.dram_tensor("probs", [BC, C], F32, kind="ExternalInput").ap()
    targ_d = nc.dram_tensor("targets", [BC], I32, kind="ExternalInput").ap()
    out_d = nc.dram_tensor("out_all", [1, OUT_W], F32, kind="ExternalOutput").ap()

    with tile.TileContext(nc) as tc:
        with ExitStack() as ctx:
            emit_kernel(ctx, tc, probs_d, targ_d, out_d)
    if split_waits:
        _split_multi_waits(nc)
    _compact_sem_ids(nc)
    _cached_nc[split_waits] = nc
    return nc


def make_in_maps(probs, targets):
    probs = np.ascontiguousarray(np.asarray(probs), dtype=np.float32)
    targets = np.asarray(targets).astype(np.int32)
    assert probs.shape == (B, C) and targets.shape == (B,)
    return [
        {
            "probs": probs[k * BC:(k + 1) * BC],
            "targets": np.ascontiguousarray(targets[k * BC:(k + 1) * BC]),
        }
        for k in range(NCORES)
    ]


def combine(results):
    cs = np.zeros(C, np.float64)
    hs = np.zeros(C, np.float64)
    fc = 0.0
    for r in results:
        row = r["out_all"].reshape(OUT_W).astype(np.float64)
        cs += row[0:C]
        hs += row[C:2 * C]
        fc += row[2 * C]
    loss_cls = fc / B
    loss_cal = float(np.mean(np.abs(cs / B - hs / B)))
    return np.asarray(loss_cls + 1.0 * loss_cal, dtype=np.float32)


def run_spmd(probs, targets, **kwargs):
    nc = build_nc()
    in_maps = make_in_maps(probs, targets)
    return run_bass_kernel_spmd(nc, in_maps, list(range(NCORES)), **kwargs)


def kernel(probs, targets):
    res = run_spmd(probs, targets)
    return combine(res.results)
